# revision 13
# baseline (speedup 1.0000x reference)
"""Trainium2 Bass kernel for nn_Attention (B=4, N=2048, C=768, H=8).

reference:
    qkv = x.reshape(B,N,H,d).transpose(0,2,1,3)      # q=k=v
    attn = softmax(q @ k^T / sqrt(d))
    out  = (attn @ v).transpose -> (B,N,C)
    y    = out @ proj_w.T + proj_b

Sharding: 8 cores = 4 batches x 2 head-groups of 4 heads.  Each core
computes full 2048-query attention for its 4 heads plus the partial
projection over its 384 channels; the host gather sums the two partial
y^T tensors per batch (the unshard step for head-sharded partials) --
bias is folded into the even core's input data.

Key trick: q = k, so each head's 2048x2048 score matrix is symmetric and
fully core-local.  Scores are computed only for the 40 of 64 (128x512)
tiles on or below the diagonal; the 24 tiles above it are reconstructed
after exp by DMA-XBAR transposes of the mirror tiles (exp preserves
symmetry), saving 37.5%% of both the score matmuls (PE) and the exp
stream (ScalarE).

Layout is transposed ([feature, token]) as in the q-half-sharded
variant: per head, es supertiles [128 key, 4 ktile, 512 q] hold
exp(scale*S^T); supertile (q,g) with g>q is filled by 4 DMA transposes
from computed supertile (g,q).  PV contracts [vn | 1] against es
(row 96 = softmax denominator), normalization runs on DVE+GpSimd with a
DMA repack into 128-row c-chunks, and the projection accumulates the 3
local c-chunks into PSUM per output j-tile.

Schedule: software pipeline over heads -- unit k interleaves the score
matmuls + exps of head h with the PV/normalize (and, at iteration
boundaries, projection) work of head h-1, so the PE stream stays dense
(HAM pstate defense) while the ScalarE exp stream runs just below PE
occupancy.  PSUM: 2x2-bank score buffers + 2 PV accumulators + 2
projection accumulators = 8 banks.
"""

import sys
import os

for _p in ("/opt/trn_rl_repo",):
    if os.path.isdir(_p) and _p not in sys.path:
        sys.path.insert(0, _p)

import numpy as np
import ml_dtypes

import concourse.bacc as bacc
import concourse.mybir as mybir
import concourse.tile as tile
from concourse.bass import MemorySpace
from concourse.bass_utils import run_bass_kernel_spmd

BF16 = ml_dtypes.bfloat16

B, N, C = 4, 2048, 768
H = 8
NH = 4                # local heads per core
D = C // H            # 96
NCORES = 8
QC = 512              # q chunk (PSUM free size)
NQC = N // QC         # 4
KT = N // 128         # 16 key tiles
NG = KT // 4          # 4 ktile groups (supertile columns)
JT = C // 128         # 6 output-feature tiles
CT = NH * D // 128    # 3 local c-chunks
SCALE = float(D) ** -0.5

_cache = {}


def build_bass(iters: int = 1):
    """Build the SPMD single-core program (same graph on all 8 cores)."""
    nc = bacc.Bacc("TRN2", target_bir_lowering=False, debug=False,
                   num_devices=NCORES)
    f32 = mybir.dt.float32
    bf16 = mybir.dt.bfloat16

    kt = nc.declare_dram_parameter("kt", [NH, D, N], bf16, isOutput=False)
    vn = nc.declare_dram_parameter("vn", [N, NH, D + 1], bf16, isOutput=False)
    wt = nc.declare_dram_parameter("wt", [CT, 128, C], bf16, isOutput=False)
    bias = nc.declare_dram_parameter("bias", [JT, 128, 1], f32, isOutput=False)
    out = nc.declare_dram_parameter("out", [C, N], f32, isOutput=True)

    with tile.TileContext(nc) as tc:
        with (
            tc.tile_pool(name="consts", bufs=1) as consts,
            tc.tile_pool(name="esp", bufs=22) as esp,
            tc.tile_pool(name="espt", bufs=8) as espt,
            tc.tile_pool(name="ogp", bufs=2 * NQC * CT) as ogp,
            tc.tile_pool(name="small", bufs=3) as small,
            tc.tile_pool(name="ysb", bufs=3) as ysbp,
            tc.tile_pool(name="ps_s", bufs=2, space=MemorySpace.PSUM) as ps_s,
            tc.tile_pool(name="ps_o", bufs=2, space=MemorySpace.PSUM) as ps_o,
            tc.tile_pool(name="ps_y", bufs=2, space=MemorySpace.PSUM) as ps_y,
        ):
            # ---- load constants (first-needed first) ----
            kt_sb = [consts.tile([D, N], bf16, tag=f"kt{h}", name=f"kt{h}")
                     for h in range(NH)]
            vn_sb = [consts.tile([128, NH, D + 1], bf16, tag=f"vn{t}",
                                 name=f"vn{t}") for t in range(KT)]
            wt_sb = [consts.tile([128, C], bf16, tag=f"wt{c}", name=f"wt{c}")
                     for c in range(CT)]
            bias_sb = [consts.tile([128, 1], f32, tag=f"bias{j}",
                                   name=f"bias{j}") for j in range(JT)]
            nc.sync.dma_start(out=kt_sb[0][:], in_=kt[0])
            nc.sync.dma_start(out=kt_sb[1][:], in_=kt[1])
            for t in range(KT):
                nc.sync.dma_start(out=vn_sb[t][:],
                                  in_=vn[t * 128:(t + 1) * 128])
            nc.sync.dma_start(out=kt_sb[2][:], in_=kt[2])
            nc.sync.dma_start(out=kt_sb[3][:], in_=kt[3])
            for c in range(CT):
                nc.sync.dma_start(out=wt_sb[c][:], in_=wt[c])
            for j in range(JT):
                nc.sync.dma_start(out=bias_sb[j][:], in_=bias[j])

            # HAM warmup: dummy matmuls with no input deps keep the PE
            # activity monitor busy during the initial DMA wait, and an
            # exp preloads the ScalarE table set.
            wz = consts.tile([D, QC], bf16, tag="wz", name="wz")
            nc.vector.memset(wz[:], 0)
            pyw = ps_y.tile([128, QC], f32, tag="py", name="pyw")
            for _w in range(8):
                nc.tensor.matmul(pyw[:], wz[:, 0:128], wz[:],
                                 start=True, stop=True)
            wze = small.tile([1, 16], bf16, tag="wze", name="wze")
            nc.scalar.activation(out=wze[:], in_=wz[0:1, 0:16],
                                 func=mybir.ActivationFunctionType.Exp)

            og_gen = {}       # it -> {(qc, c) -> og tile}

            def make_a_groups(h):
                """A(h): per qc, mm1+exp over computed ktiles in pairs."""
                groups = []
                for qc in range(NQC):
                    for pair in range(2 * (qc + 1)):
                        groups.append((h, qc, pair))
                return groups

            cur_est = [None]  # supertile registry for the A-head in flight

            def emit_a_group(job):
                h, qc, pair = job
                g = pair // 2
                half = pair % 2
                if half == 0:
                    st = esp.tile([128, 4, QC], bf16, tag="es",
                                  name=f"es{qc}_{g}")
                    cur_est[0][(qc, g)] = st
                st = cur_est[0][(qc, g)]
                ps = ps_s.tile([128, 2, QC], f32, tag="ps", name="ps")
                for i in range(2):
                    t = 2 * pair + i
                    nc.tensor.matmul(
                        ps[:, i, :],
                        kt_sb[h][:, t * 128:(t + 1) * 128],
                        kt_sb[h][:, qc * QC:(qc + 1) * QC],
                        start=True, stop=True,
                    )
                nc.scalar.activation(
                    out=st[:, 2 * half:2 * half + 2, :],
                    in_=ps[:, 0:2, :],
                    func=mybir.ActivationFunctionType.Exp,
                    scale=SCALE,
                )

            def emit_transposes(prev_est):
                # Skipped supertile (q,g), g>q, filled from computed (g,q)
                # in ONE xbar transpose: the [128, 4, 4, 128] layout
                # (colblock-major, then ktile) flattens to exactly the
                # [128, 16, 128] transpose of the source's [128, 2048]
                # view.  Ordered so the B-sweep (qc 3->0) sees its tiles
                # just in time: (2,3) first (source = last A supertile).
                order = [(2, 3), (1, 2), (1, 3), (0, 1), (0, 2), (0, 3)]
                for q, g in order:
                    src = prev_est[(g, q)]
                    dst = espt.tile([128, 4, 4, 128], bf16, tag="esT",
                                    name=f"esT{q}_{g}")
                    prev_est[(q, g)] = dst
                    nc.sync.dma_start(out=dst[:], in_=src[:],
                                      transpose=True)

            def make_b_queue(h_prev, prev_est, it_b, with_proj):
                """B(it_b, h_prev): PV + normalize per qc; with_proj also
                emits the projection of iteration it_b per qc (only valid
                when h_prev == NH-1, i.e. the generation completes here)."""
                bq = []
                po_box = {}
                og = og_gen.setdefault(it_b, {})

                def mk_pv(qc, t):
                    def go():
                        if t == 0:
                            po_box[qc] = ps_o.tile([D + 1, QC], f32, tag="po",
                                                   name="po")
                        v = t // 4
                        st = prev_est[(qc, v)]
                        if v <= qc:
                            rhs = st[:, t % 4, :]
                        else:
                            rhs = st[:, :, t % 4, :]
                        nc.tensor.matmul(
                            po_box[qc][:],
                            vn_sb[t][:, h_prev, :],
                            rhs,
                            start=(t == 0), stop=(t == KT - 1),
                        )
                    return go

                def mk_norm(qc):
                    def go():
                        po = po_box[qc]
                        if h_prev == 0:
                            for c in range(CT):
                                og[(qc, c)] = ogp.tile(
                                    [128, QC], bf16, tag="og",
                                    name=f"og{qc}_{c}")
                        oc = small.tile([D + 1, QC], f32, tag="oc", name="oc")
                        nc.vector.tensor_copy(out=oc[:], in_=po[:])
                        rc = small.tile([1, QC], f32, tag="rc", name="rc")
                        nc.vector.reciprocal(out=rc[:], in_=oc[D:D + 1, :])
                        bc = small.tile([D, QC], f32, tag="bc", name="bc")
                        nc.gpsimd.partition_broadcast(bc[:], rc[:])
                        on = small.tile([D, QC], bf16, tag="on", name="on")
                        nc.vector.tensor_mul(on[:], oc[0:D, :], bc[:])
                        a0 = (D * h_prev) % 128
                        c0 = (D * h_prev) // 128
                        s1 = min(128 - a0, D)
                        nc.gpsimd.dma_start(out=og[(qc, c0)][a0:a0 + s1, :],
                                            in_=on[0:s1, :])
                        if s1 < D:
                            nc.gpsimd.dma_start(
                                out=og[(qc, c0 + 1)][0:D - s1, :],
                                in_=on[s1:D, :])
                    return go

                def mk_proj_mm(qc, j, c, py_box):
                    def go():
                        if c == 0:
                            py_box[0] = ps_y.tile([128, QC], f32, tag="py",
                                                  name="py")
                        nc.tensor.matmul(
                            py_box[0][:],
                            wt_sb[c][:, j * 128:(j + 1) * 128],
                            og[(qc, c)][:],
                            start=(c == 0), stop=(c == CT - 1),
                        )
                    return go

                def mk_proj_fin(qc, j, py_box):
                    def go():
                        y = ysbp.tile([128, QC], f32, tag="y", name="y")
                        nc.vector.tensor_scalar_add(
                            out=y[:], in0=py_box[0][:],
                            scalar1=bias_sb[j][:],
                        )
                        nc.sync.dma_start(
                            out=out[j * 128:(j + 1) * 128,
                                    qc * QC:(qc + 1) * QC],
                            in_=y[:],
                        )
                    return go

                for qc in range(NQC - 1, -1, -1):
                    for t in range(KT):
                        bq.append(mk_pv(qc, t))
                    bq.append(mk_norm(qc))
                    if with_proj:
                        for j in range(JT):
                            py_box = [None]
                            for c in range(CT):
                                bq.append(mk_proj_mm(qc, j, c, py_box))
                            bq.append(mk_proj_fin(qc, j, py_box))
                return bq

            def emit_slot(aq, bq):
                """Interleave a-groups with B work so PE stays dense."""
                na = len(aq)
                for idx, job in enumerate(aq):
                    emit_a_group(job)
                    rem_a = na - idx - 1
                    take = len(bq) if rem_a == 0 else -(-len(bq) // (rem_a + 1))
                    for _ in range(take):
                        bq.pop(0)()
                while bq:
                    bq.pop(0)()

            units = [(it, h) for it in range(iters) for h in range(NH)]
            prev = None  # (h_prev, est_prev, it_prev)
            for it, h in units:
                cur_est[0] = {}
                aq = make_a_groups(h)
                if prev is None:
                    emit_slot(aq, [])
                else:
                    h_prev, est_prev, it_prev = prev
                    emit_transposes(est_prev)
                    with_proj = (h_prev == NH - 1)
                    bq = make_b_queue(h_prev, est_prev, it_prev, with_proj)
                    emit_slot(aq, bq)
                    if with_proj:
                        og_gen.pop(it_prev, None)
                prev = (h, cur_est[0], it)

            # drain: B(last head) + projection of the last iteration
            h_prev, est_prev, it_prev = prev
            emit_transposes(est_prev)
            bq = make_b_queue(h_prev, est_prev, it_prev, True)
            emit_slot([], bq)
    nc.compile()
    return nc


def shard_inputs(x, proj_w, proj_b):
    x = np.asarray(x, dtype=np.float32)
    proj_w = np.asarray(proj_w, dtype=np.float32)
    proj_b = np.asarray(proj_b, dtype=np.float32)

    bias_full = np.ascontiguousarray(proj_b).reshape(JT, 128, 1)
    bias_zero = np.zeros((JT, 128, 1), dtype=np.float32)

    in_maps = []
    for c in range(NCORES):
        b = c // 2
        g = c % 2
        xb = x[b]                                   # (N, C)
        xtb = np.ascontiguousarray(xb.T)            # (C, N)
        kt_c = np.ascontiguousarray(
            xtb.reshape(H, D, N)[NH * g:NH * (g + 1)]).astype(BF16)
        vn_f = np.ones((N, NH, D + 1), dtype=np.float32)
        vn_f[:, :, :D] = xb.reshape(N, H, D)[:, NH * g:NH * (g + 1), :]
        wt_c = np.ascontiguousarray(
            proj_w[:, 384 * g:384 * (g + 1)].T).reshape(CT, 128, C)
        in_maps.append({
            "kt": kt_c,
            "vn": vn_f.astype(BF16),
            "wt": wt_c.astype(BF16),
            "bias": bias_full if g == 0 else bias_zero,
        })
    return in_maps


def assemble(results):
    y = np.empty((B, N, C), dtype=np.float32)
    for b in range(B):
        y[b] = (results[2 * b]["out"] + results[2 * b + 1]["out"]).T
    return y


def kernel(x, proj_w, proj_b):
    if "nc" not in _cache:
        _cache["nc"] = build_bass()
    nc = _cache["nc"]
    in_maps = shard_inputs(x, proj_w, proj_b)
    res = run_bass_kernel_spmd(nc, in_maps, core_ids=list(range(NCORES)))
    return assemble(res.results)


# revision 14
# speedup vs baseline: 1.5690x; 1.5690x over previous
"""Trainium2 Bass kernel for nn_Attention (B=4, N=2048, C=768, H=8).

reference:
    qkv = x.reshape(B,N,H,d).transpose(0,2,1,3)      # q=k=v
    attn = softmax(q @ k^T / sqrt(d))
    out  = (attn @ v).transpose -> (B,N,C)
    y    = out @ proj_w.T + proj_b

Sharding: 8 cores = 4 batches x 2 head-groups of 4 heads.  Each core
computes full 2048-query attention for its 4 heads plus the partial
projection over its 384 channels; the host gather sums the two partial
y^T tensors per batch (the unshard step for head-sharded partials) --
bias is folded into the even core's input data.

Key trick: q = k, so each head's 2048x2048 score matrix is symmetric and
fully core-local.  Scores are computed only for the 40 of 64 (128x512)
tiles on or below the diagonal; the 24 tiles above it are reconstructed
after exp by DMA-XBAR transposes of the mirror tiles (exp preserves
symmetry), saving 37.5%% of both the score matmuls (PE) and the exp
stream (ScalarE).

Layout is transposed ([feature, token]) as in the q-half-sharded
variant: per head, es supertiles [128 key, 4 ktile, 512 q] hold
exp(scale*S^T); supertile (q,g) with g>q is filled by 4 DMA transposes
from computed supertile (g,q).  PV contracts [vn | 1] against es
(row 96 = softmax denominator), normalization runs on DVE+GpSimd with a
DMA repack into 128-row c-chunks, and the projection accumulates the 3
local c-chunks into PSUM per output j-tile.

Schedule: software pipeline over heads -- unit k interleaves the score
matmuls + exps of head h with the PV/normalize (and, at iteration
boundaries, projection) work of head h-1, so the PE stream stays dense
(HAM pstate defense) while the ScalarE exp stream runs just below PE
occupancy.  PSUM: 2x2-bank score buffers + 2 PV accumulators + 2
projection accumulators = 8 banks.
"""

import sys
import os

for _p in ("/opt/trn_rl_repo",):
    if os.path.isdir(_p) and _p not in sys.path:
        sys.path.insert(0, _p)

import numpy as np
import ml_dtypes

import concourse.bacc as bacc
import concourse.mybir as mybir
import concourse.tile as tile
from concourse.bass import MemorySpace
from concourse.bass_utils import run_bass_kernel_spmd

BF16 = ml_dtypes.bfloat16

B, N, C = 4, 2048, 768
H = 8
NH = 4                # local heads per core
D = C // H            # 96
NCORES = 8
QC = 512              # q chunk (PSUM free size)
NQC = N // QC         # 4
KT = N // 128         # 16 key tiles
NG = KT // 4          # 4 ktile groups (supertile columns)
JT = C // 128         # 6 output-feature tiles
CT = NH * D // 128    # 3 local c-chunks
SCALE = float(D) ** -0.5

_cache = {}


def build_bass(iters: int = 1):
    """Build the SPMD single-core program (same graph on all 8 cores)."""
    nc = bacc.Bacc("TRN2", target_bir_lowering=False, debug=False,
                   num_devices=NCORES)
    f32 = mybir.dt.float32
    bf16 = mybir.dt.bfloat16

    kt = nc.declare_dram_parameter("kt", [NH, D, N], bf16, isOutput=False)
    vn = nc.declare_dram_parameter("vn", [N, NH, D + 1], bf16, isOutput=False)
    wt = nc.declare_dram_parameter("wt", [CT, 128, C], bf16, isOutput=False)
    bias = nc.declare_dram_parameter("bias", [JT, 128, 1], f32, isOutput=False)
    out = nc.declare_dram_parameter("out", [C, N], f32, isOutput=True)

    with tile.TileContext(nc) as tc:
        with (
            tc.tile_pool(name="consts", bufs=1) as consts,
            tc.tile_pool(name="esp", bufs=22) as esp,
            tc.tile_pool(name="espt", bufs=8) as espt,
            tc.tile_pool(name="ogp", bufs=2 * NQC * CT) as ogp,
            tc.tile_pool(name="small", bufs=3) as small,
            tc.tile_pool(name="ysb", bufs=3) as ysbp,
            tc.tile_pool(name="ps_s", bufs=2, space=MemorySpace.PSUM) as ps_s,
            tc.tile_pool(name="ps_o", bufs=2, space=MemorySpace.PSUM) as ps_o,
            tc.tile_pool(name="ps_y", bufs=2, space=MemorySpace.PSUM) as ps_y,
        ):
            # ---- load constants (first-needed first) ----
            kt_sb = [consts.tile([D, N], bf16, tag=f"kt{h}", name=f"kt{h}")
                     for h in range(NH)]
            vn_sb = [consts.tile([128, NH, D + 1], bf16, tag=f"vn{t}",
                                 name=f"vn{t}") for t in range(KT)]
            wt_sb = [consts.tile([128, C], bf16, tag=f"wt{c}", name=f"wt{c}")
                     for c in range(CT)]
            bias_sb = [consts.tile([128, 1], f32, tag=f"bias{j}",
                                   name=f"bias{j}") for j in range(JT)]
            nc.sync.dma_start(out=kt_sb[0][:], in_=kt[0])
            nc.sync.dma_start(out=kt_sb[1][:], in_=kt[1])
            for t in range(KT):
                nc.sync.dma_start(out=vn_sb[t][:],
                                  in_=vn[t * 128:(t + 1) * 128])
            nc.sync.dma_start(out=kt_sb[2][:], in_=kt[2])
            nc.sync.dma_start(out=kt_sb[3][:], in_=kt[3])
            for c in range(CT):
                nc.sync.dma_start(out=wt_sb[c][:], in_=wt[c])
            for j in range(JT):
                nc.sync.dma_start(out=bias_sb[j][:], in_=bias[j])

            # HAM warmup: dummy matmuls with no input deps keep the PE
            # activity monitor busy during the initial DMA wait, and an
            # exp preloads the ScalarE table set.
            wz = consts.tile([D, QC], bf16, tag="wz", name="wz")
            nc.vector.memset(wz[:], 0)
            pyw = ps_y.tile([128, QC], f32, tag="py", name="pyw")
            for _w in range(8):
                nc.tensor.matmul(pyw[:], wz[:, 0:128], wz[:],
                                 start=True, stop=True)
            wze = small.tile([1, 16], bf16, tag="wze", name="wze")
            nc.scalar.activation(out=wze[:], in_=wz[0:1, 0:16],
                                 func=mybir.ActivationFunctionType.Exp)

            og_gen = {}       # it -> {(qc, c) -> og tile}

            def make_a_groups(h):
                """A(h): per qc, mm1+exp over computed ktiles in pairs."""
                groups = []
                for qc in range(NQC):
                    for pair in range(2 * (qc + 1)):
                        groups.append((h, qc, pair))
                return groups

            cur_est = [None]  # supertile registry for the A-head in flight

            def emit_a_group(job):
                h, qc, pair = job
                g = pair // 2
                half = pair % 2
                if half == 0:
                    st = esp.tile([128, 4, QC], bf16, tag="es",
                                  name=f"es{qc}_{g}")
                    cur_est[0][(qc, g)] = st
                st = cur_est[0][(qc, g)]
                ps = ps_s.tile([128, 2, QC], f32, tag="ps", name="ps")
                for i in range(2):
                    t = 2 * pair + i
                    nc.tensor.matmul(
                        ps[:, i, :],
                        kt_sb[h][:, t * 128:(t + 1) * 128],
                        kt_sb[h][:, qc * QC:(qc + 1) * QC],
                        start=True, stop=True,
                    )
                nc.scalar.activation(
                    out=st[:, 2 * half:2 * half + 2, :],
                    in_=ps[:, 0:2, :],
                    func=mybir.ActivationFunctionType.Exp,
                    scale=SCALE,
                )
                # supertile (qc, g) complete -> if strictly below the
                # diagonal, transpose-fill its mirror (g, qc) right away so
                # the xbar DMAs overlap this slot's compute.
                if half == 1 and g < qc:
                    dst = espt.tile([128, 4, 4, 128], bf16, tag="esT",
                                    name=f"esT{g}_{qc}")
                    cur_est[0][(g, qc)] = dst
                    nc.sync.dma_start(out=dst[:], in_=st[:],
                                      transpose=True)

            def emit_transposes(prev_est):
                # Skipped supertile (q,g), g>q, filled from computed (g,q)
                # in ONE xbar transpose: the [128, 4, 4, 128] layout
                # (colblock-major, then ktile) flattens to exactly the
                # [128, 16, 128] transpose of the source's [128, 2048]
                # view.  Ordered so the B-sweep (qc 3->0) sees its tiles
                # just in time: (2,3) first (source = last A supertile).
                order = [(2, 3), (1, 2), (1, 3), (0, 1), (0, 2), (0, 3)]
                for q, g in order:
                    src = prev_est[(g, q)]
                    dst = espt.tile([128, 4, 4, 128], bf16, tag="esT",
                                    name=f"esT{q}_{g}")
                    prev_est[(q, g)] = dst
                    nc.sync.dma_start(out=dst[:], in_=src[:],
                                      transpose=True)

            def make_b_queue(h_prev, prev_est, it_b, with_proj):
                """B(it_b, h_prev): PV + normalize per qc; with_proj also
                emits the projection of iteration it_b per qc (only valid
                when h_prev == NH-1, i.e. the generation completes here)."""
                bq = []
                po_box = {}
                og = og_gen.setdefault(it_b, {})

                def mk_pv(qc, t):
                    def go():
                        if t == 0:
                            po_box[qc] = ps_o.tile([D + 1, QC], f32, tag="po",
                                                   name="po")
                        v = t // 4
                        st = prev_est[(qc, v)]
                        if v <= qc:
                            rhs = st[:, t % 4, :]
                        else:
                            rhs = st[:, :, t % 4, :]
                        nc.tensor.matmul(
                            po_box[qc][:],
                            vn_sb[t][:, h_prev, :],
                            rhs,
                            start=(t == 0), stop=(t == KT - 1),
                        )
                    return go

                def mk_norm(qc):
                    def go():
                        po = po_box[qc]
                        if h_prev == 0:
                            for c in range(CT):
                                og[(qc, c)] = ogp.tile(
                                    [128, QC], bf16, tag="og",
                                    name=f"og{qc}_{c}")
                        oc = small.tile([D + 1, QC], f32, tag="oc", name="oc")
                        nc.vector.tensor_copy(out=oc[:], in_=po[:])
                        rc = small.tile([1, QC], f32, tag="rc", name="rc")
                        nc.vector.reciprocal(out=rc[:], in_=oc[D:D + 1, :])
                        bc = small.tile([D, QC], f32, tag="bc", name="bc")
                        nc.gpsimd.partition_broadcast(bc[:], rc[:])
                        on = small.tile([D, QC], bf16, tag="on", name="on")
                        nc.vector.tensor_mul(on[:], oc[0:D, :], bc[:])
                        a0 = (D * h_prev) % 128
                        c0 = (D * h_prev) // 128
                        s1 = min(128 - a0, D)
                        nc.gpsimd.dma_start(out=og[(qc, c0)][a0:a0 + s1, :],
                                            in_=on[0:s1, :])
                        if s1 < D:
                            nc.gpsimd.dma_start(
                                out=og[(qc, c0 + 1)][0:D - s1, :],
                                in_=on[s1:D, :])
                    return go

                def mk_proj_mm(qc, j, c, py_box):
                    def go():
                        if c == 0:
                            py_box[0] = ps_y.tile([128, QC], f32, tag="py",
                                                  name="py")
                        nc.tensor.matmul(
                            py_box[0][:],
                            wt_sb[c][:, j * 128:(j + 1) * 128],
                            og[(qc, c)][:],
                            start=(c == 0), stop=(c == CT - 1),
                        )
                    return go

                def mk_proj_fin(qc, j, py_box):
                    def go():
                        y = ysbp.tile([128, QC], f32, tag="y", name="y")
                        nc.vector.tensor_scalar_add(
                            out=y[:], in0=py_box[0][:],
                            scalar1=bias_sb[j][:],
                        )
                        nc.sync.dma_start(
                            out=out[j * 128:(j + 1) * 128,
                                    qc * QC:(qc + 1) * QC],
                            in_=y[:],
                        )
                    return go

                for qc in range(NQC - 1, -1, -1):
                    for t in range(KT):
                        bq.append(mk_pv(qc, t))
                    bq.append(mk_norm(qc))
                    if with_proj:
                        for j in range(JT):
                            py_box = [None]
                            for c in range(CT):
                                bq.append(mk_proj_mm(qc, j, c, py_box))
                            bq.append(mk_proj_fin(qc, j, py_box))
                return bq

            def emit_slot(aq, bq):
                """Interleave a-groups with B work so PE stays dense."""
                na = len(aq)
                for idx, job in enumerate(aq):
                    emit_a_group(job)
                    rem_a = na - idx - 1
                    take = len(bq) if rem_a == 0 else -(-len(bq) // (rem_a + 1))
                    for _ in range(take):
                        bq.pop(0)()
                while bq:
                    bq.pop(0)()

            units = [(it, h) for it in range(iters) for h in range(NH)]
            prev = None  # (h_prev, est_prev, it_prev)
            for it, h in units:
                cur_est[0] = {}
                aq = make_a_groups(h)
                if prev is None:
                    emit_slot(aq, [])
                else:
                    h_prev, est_prev, it_prev = prev
                    emit_transposes(est_prev)
                    with_proj = (h_prev == NH - 1)
                    bq = make_b_queue(h_prev, est_prev, it_prev, with_proj)
                    emit_slot(aq, bq)
                    if with_proj:
                        og_gen.pop(it_prev, None)
                prev = (h, cur_est[0], it)

            # drain: B(last head) + projection of the last iteration
            h_prev, est_prev, it_prev = prev
            emit_transposes(est_prev)
            bq = make_b_queue(h_prev, est_prev, it_prev, True)
            emit_slot([], bq)
    nc.compile()
    return nc


def shard_inputs(x, proj_w, proj_b):
    x = np.asarray(x, dtype=np.float32)
    proj_w = np.asarray(proj_w, dtype=np.float32)
    proj_b = np.asarray(proj_b, dtype=np.float32)

    bias_full = np.ascontiguousarray(proj_b).reshape(JT, 128, 1)
    bias_zero = np.zeros((JT, 128, 1), dtype=np.float32)

    in_maps = []
    for c in range(NCORES):
        b = c // 2
        g = c % 2
        xb = x[b]                                   # (N, C)
        xtb = np.ascontiguousarray(xb.T)            # (C, N)
        kt_c = np.ascontiguousarray(
            xtb.reshape(H, D, N)[NH * g:NH * (g + 1)]).astype(BF16)
        vn_f = np.ones((N, NH, D + 1), dtype=np.float32)
        vn_f[:, :, :D] = xb.reshape(N, H, D)[:, NH * g:NH * (g + 1), :]
        wt_c = np.ascontiguousarray(
            proj_w[:, 384 * g:384 * (g + 1)].T).reshape(CT, 128, C)
        in_maps.append({
            "kt": kt_c,
            "vn": vn_f.astype(BF16),
            "wt": wt_c.astype(BF16),
            "bias": bias_full if g == 0 else bias_zero,
        })
    return in_maps


def assemble(results):
    y = np.empty((B, N, C), dtype=np.float32)
    for b in range(B):
        y[b] = (results[2 * b]["out"] + results[2 * b + 1]["out"]).T
    return y


def kernel(x, proj_w, proj_b):
    if "nc" not in _cache:
        _cache["nc"] = build_bass()
    nc = _cache["nc"]
    in_maps = shard_inputs(x, proj_w, proj_b)
    res = run_bass_kernel_spmd(nc, in_maps, core_ids=list(range(NCORES)))
    return assemble(res.results)
